# revision 75
# baseline (speedup 1.0000x reference)
"""GQA (32q/8kv heads, RoPE, causal) TRN2 kernel v4.

Sharding: 8 cores = 2 batches x 4 kv-pairs. Core (b, kvp) owns batch b,
kv heads {2kvp, 2kvp+1}, q heads 8kvp..8kvp+7. Each core emits a partial
o [S, D] (bf16); host sums 4 partials per batch.

Precision: QKV projection and o-proj in fp8e4 DoubleRow hi/lo 3-term
(0.75x bf16 cost, ~2^-9 rel error). Scores/probs/AV in bf16 (contraction
64 / the hi/lo slots can't double there, so fp8 buys nothing in the
ap-size cost model). o-proj operands scaled (at x16 via a 1/16 ones-col,
Wo x64) to keep fp8-hi in the normal range; the osb evict divides by 1024.

Schedule (the cost model charges matmuls output-free-size only; exp on ACT
costs 2.3x the score matmul that feeds it, so ACT is the phase-2 constraint):
  phase 1 (13 tiles): split weight/x DMAs so QKV starts ~4.4us in; QKV fp8
    DR -> ACT evicts (Copy, 1/64) -> RoPE (2 DVE muls + in-place add) ->
    PE transposes -> qt/kt; it=0 fully prescored (exp'd) under QKV.
  phase 2, it order (2,3,1,0): per head a weighted riffle interleaves
    score-pair groups ([128,2,512] psum, one wide exp each) with filler:
    AV of the previous head (65-wide DR-free accumulation into 2 slab
    banks + fused slab normalize on DVE), at-transposes, per-chunk fp8
    hi/lo splits (Pool cast + DVE sub + Pool cast), o-proj drains of the
    previous it (6 DR matmuls/step), and tiles 13-15's whole pipeline
    (deferred into the ACT-bound it=2 window, psum via the po ring).
    ACT-free its (1, 0) run last, absorbing drains; their o evictions go
    through ACT (idle there) instead of DVE; o DMA per 512-col chunk.
"""
import numpy as np
from contextlib import ExitStack

import concourse.bass as bass
from concourse import bacc
import concourse.mybir as mybir
import concourse.tile as tile
from concourse.bass_utils import run_bass_kernel_spmd
import ml_dtypes

F32 = mybir.dt.float32
BF16 = mybir.dt.bfloat16
FP8 = mybir.dt.float8e4
EXP = mybir.ActivationFunctionType.Exp
DR = mybir.MatmulPerfMode.DoubleRow

D = 2048
DH = 64
NCORES = 8
ROPE_BASE = 10000.0
MASKVAL = -240.0
EBIAS = -2.0     # probs = exp(0.125*scores - 2); cancels in normalization

_cached = {}


def build_nc(S=2048, dbg=False):
    NTT = S // 128
    NIT = S // 512
    KC = D // 128
    NH = 8
    nc = bacc.Bacc("TRN2", target_bir_lowering=False, debug=False)
    dbg_d = {}
    if dbg:
        dbg_d["d_qt"] = nc.declare_dram_parameter("d_qt", [64, NH, S], F32, isOutput=True)
        dbg_d["d_kt"] = nc.declare_dram_parameter("d_kt", [64, 2, S], F32, isOutput=True)
        dbg_d["d_vsb"] = nc.declare_dram_parameter("d_vsb", [128, 2, NTT, 65], F32, isOutput=True)
        dbg_d["d_at"] = nc.declare_dram_parameter("d_at", [128, 4, S], F32, isOutput=True)
        dbg_d["d_pb"] = nc.declare_dram_parameter("d_pb", [128, NTT, 512], F32, isOutput=True)
    xt8 = nc.declare_dram_parameter("xt8", [NTT, 128, KC, 2, 128], FP8, isOutput=False)
    wall8 = nc.declare_dram_parameter("wall8", [128, KC, 2, 768], FP8, isOutput=False)
    wothb = nc.declare_dram_parameter("wothb", [128, 4, D], FP8, isOutput=False)
    wotlb = nc.declare_dram_parameter("wotlb", [128, 4, D], FP8, isOutput=False)
    cosb = nc.declare_dram_parameter("cosb", [128, NTT, 64], BF16, isOutput=False)
    sinsg = nc.declare_dram_parameter("sinsg", [128, NTT, 64], BF16, isOutput=False)
    identf = nc.declare_dram_parameter("identf", [128, 128], F32, isOutput=False)
    trif = nc.declare_dram_parameter("trif", [128, 128], F32, isOutput=False)
    o = nc.declare_dram_parameter("o", [S, D], BF16, isOutput=True)

    with tile.TileContext(nc) as tc, ExitStack() as ctx:
        wp = ctx.enter_context(tc.tile_pool(name="weights", bufs=1))
        sp = ctx.enter_context(tc.tile_pool(name="state", bufs=1))
        xs = ctx.enter_context(tc.tile_pool(name="xstream", bufs=4))
        rp = ctx.enter_context(tc.tile_pool(name="ring", bufs=2))
        pr = ctx.enter_context(tc.tile_pool(name="probs", bufs=3))
        aq = ctx.enter_context(tc.tile_pool(name="atq", bufs=2))
        ob = ctx.enter_context(tc.tile_pool(name="osb", bufs=2))
        sm = ctx.enter_context(tc.tile_pool(name="small", bufs=2))

        # ---------- persistent weights / tables ----------
        wall = wp.tile([128, KC, 2, 768], FP8, tag="wall")
        woth = wp.tile([128, 4, D], FP8, tag="woth")
        wotl = wp.tile([128, 4, D], FP8, tag="wotl")
        cos_sb = wp.tile([128, NTT, 64], BF16, tag="cos")
        sin_sb = wp.tile([128, NTT, 64], BF16, tag="sin")
        idb = wp.tile([128, 128], BF16, tag="idb")
        trib = wp.tile([128, 128], BF16, tag="trib")
        nbias = wp.tile([128, 1], F32, tag="nbias")

        # DMA issue order == execution order (single queue): x tiles first so
        # QKV can start ~4us in, then tables (needed by tile-0 rope), then
        # wall in per-chunk-pair pieces (matmul kp waits only on its chunk),
        # wot split across the loop (first needed ~90us in).
        xtiles = {}

        def xfetch(tt):
            t = xs.tile([128, KC, 2, 128], FP8, tag="xt", name=f"xt{tt}")
            nc.sync.dma_start(t[:], xt8[tt])
            xtiles[tt] = t

        def wfetch(kp):
            nc.sync.dma_start(wall[:, 2 * kp:2 * kp + 2, :, :],
                              wall8[:, 2 * kp:2 * kp + 2, :, :])

        xfetch(0)
        wfetch(0)
        wfetch(1)
        xfetch(1)
        nc.sync.dma_start(cos_sb[:], cosb[:, :, :])
        nc.sync.dma_start(sin_sb[:], sinsg[:, :, :])
        wfetch(2)
        idf_s = sm.tile([128, 128], F32, tag="idf")
        trf_s = sm.tile([128, 128], F32, tag="trf")
        nc.sync.dma_start(idf_s[:], identf[:, :])
        nc.sync.dma_start(trf_s[:], trif[:, :])
        for kp in range(3, KC // 2):
            wfetch(kp)
        nc.vector.tensor_copy(idb[:], idf_s[:])
        nc.vector.tensor_copy(trib[:], trf_s[:])
        nc.vector.memset(nbias[:], EBIAS)

        # ---------- per-core state ----------
        qt = sp.tile([64, NH, S], BF16, tag="qt")
        kt = sp.tile([64, 2, S], BF16, tag="kt")
        vsb = sp.tile([128, 2, NTT, 65], BF16, tag="vsb")
        # at held as fp8 hi/lo (o-proj runs in fp8 DoubleRow); atq carries
        # 16*at (ones-col = 1/16 makes rec = 16/denom) so the fp8-hi of small
        # at entries stays in the normal range; o eviction divides by 16*64.
        ath = sp.tile([128, 4, S], FP8, tag="ath")
        atl = sp.tile([128, 4, S], FP8, tag="atl")
        nc.vector.memset(vsb[:, :, :, 64:65], 1.0 / 16.0)

        # ================= phase 1: QKV + rope + transposes =================
        pq_pool = ExitStack()
        pp_qkv = pq_pool.enter_context(tc.tile_pool(name="pqkv", bufs=2, space="PSUM"))
        pp_tr = pq_pool.enter_context(tc.tile_pool(name="ptr", bufs=2, space="PSUM"))
        pre_pool = ExitStack()
        pp_pre = pre_pool.enter_context(tc.tile_pool(name="presc", bufs=1, space="PSUM"))

        pbs = [None] * NH
        pre_pbs = {}
        prescored = set()

        def score_group_thunks(it, h, pool, scbufs, paired, only="all",
                               ptag="sc2"):
            """Allocate pb for (it, h), return one thunk per psum score group
            (off-diag pair or diag single). only="off"/"diag" emits just that
            subset ("diag" reuses the pb allocated by an earlier "off" call —
            partial prescoring)."""
            kv = h // 4
            nkb = 4 * it + 4
            i0 = it * 512
            if only == "diag":
                pb = pre_pbs[(it, h)]
                pbs[h] = pb
            else:
                tag = "probs0" if (it == 0 and NTT >= 16) else "probs"
                pbufs = 6 if tag == "probs0" else 2
                pb = pr.tile([128, nkb, 512], BF16, tag=tag, bufs=pbufs,
                             name=f"pb{it}_{h}")
                pbs[h] = pb
                pre_pbs[(it, h)] = pb
            ndiag = 4 * it
            if paired:
                groups = []
                if only in ("all", "off"):
                    groups += [(kb, kb + 1) for kb in range(0, ndiag, 2)]
                if only in ("all", "diag"):
                    groups += [(kb,) for kb in range(ndiag, nkb)]
            else:
                groups = [(kb,) for kb in range(nkb)]

            def emit_group(grp):
                if len(grp) == 2:
                    # off-diagonal pair: 2 one-bank psum slots, one wide exp
                    p2 = pool.tile([128, 2, 512], F32, tag=ptag, bufs=scbufs,
                                   name=f"psc{it}_{h}_{grp[0]}p")
                    for j, kb in enumerate(grp):
                        nc.tensor.matmul(p2[:, j, :],
                                         kt[:, kv, kb * 128:(kb + 1) * 128],
                                         qt[:, h, i0:i0 + 512],
                                         start=True, stop=True)
                    nc.scalar.activation(pb[:, grp[0]:grp[0] + 2, :], p2[:],
                                         EXP, scale=0.125, bias=nbias[:])
                    return
                kb = grp[0]
                diag = kb >= ndiag
                qlo = (kb - ndiag) * 128 if diag else 0
                psc = pool.tile([128, 512], F32, tag="sc2", bufs=scbufs,
                                name=f"psc{it}_{h}_{kb}")
                nc.tensor.matmul(psc[:, qlo:512],
                                 kt[:, kv, kb * 128:(kb + 1) * 128],
                                 qt[:, h, i0 + qlo:i0 + 512],
                                 start=True, stop=not diag)
                if diag:
                    nc.tensor.matmul(psc[:, qlo:qlo + 128], idb[:], trib[:],
                                     start=False, stop=True)
                nc.scalar.activation(pb[:, kb, qlo:512], psc[:, qlo:512],
                                     EXP, scale=0.125, bias=nbias[:])
                if qlo:
                    nc.gpsimd.memset(pb[:, kb, 0:qlo], 0.0)

            return [lambda grp=grp: emit_group(grp) for grp in groups]

        def emit_scores(it, h, pool, scbufs, paired=False):
            for t in score_group_thunks(it, h, pool, scbufs, paired):
                t()

        def p1_tail(tt, qk8):
            tsl = slice(tt * 128, (tt + 1) * 128)
            qtr = pp_tr.tile([64, 8, 128], BF16, tag="qtr", name=f"qtr{tt}")
            ktr = pp_tr.tile([64, 8, 128], BF16, tag="qtr", name=f"ktr{tt}")
            ktr = ktr[:, 0:2, :]
            for h in range(8):
                nc.tensor.matmul(qtr[:, h, :], qk8[:, h * 64:(h + 1) * 64],
                                 idb[:], is_transpose=True,
                                 start=(h == 0), stop=(h == 7))
            for g in range(2):
                nc.tensor.matmul(ktr[:, g, :],
                                 qk8[:, 512 + g * 64:512 + (g + 1) * 64],
                                 idb[:], is_transpose=True,
                                 start=(g == 0), stop=(g == 1))
            nc.vector.tensor_copy(qt[:, :, tsl], qtr[:])
            nc.vector.tensor_copy(kt[:, :, tsl], ktr[:])

        # Tiles 13..15 are deferred into phase 2 (it=0 is fully prescored, so
        # the PE sits under ACT-idle there; their psum comes from the po ring).
        NDEF = 4 if NTT >= 16 else 0
        NP1 = NTT - NDEF
        PRE = {5: [(0, 0)], 6: [(0, 1)], 7: [(0, 2)], 8: [(0, 3)],
               9: [(0, 4)], 10: [(0, 5)]}
        PREP = {10: (2, 0), 11: (2, 1)}
        partials = set()

        def emit_rope(tt, qkvb):
            # rope: op1 tmp = pairswap(qk)*sinsg (Pool); op2 t1 = qk*cos
            # (Pool); op3 qk8 = t1+tmp in-place (DVE, bf16 2x); + vsb copy
            qkv = qkvb[:, 0:640]
            swp = qkv.rearrange("p (h n two) -> p h n two", two=2, n=32)[..., ::-1]
            tmp = rp.tile([128, 640], BF16, tag="tmp", name=f"tmp{tt}")
            t1 = rp.tile([128, 640], BF16, tag="t1", name=f"t1_{tt}")
            sin4 = sin_sb[:, tt, :].rearrange("p (one n two) -> p one n two",
                                              one=1, two=2).to_broadcast([128, 10, 32, 2])
            cos3 = cos_sb[:, tt, :].rearrange("p (one c) -> p one c",
                                              one=1).to_broadcast([128, 10, 64])
            nc.vector.tensor_mul(tmp[:].rearrange("p (h n two) -> p h n two",
                                                  two=2, n=32),
                                 swp, sin4)
            nc.vector.tensor_mul(t1[:].rearrange("p (h c) -> p h c", h=10),
                                 qkv.rearrange("p (h c) -> p h c", h=10), cos3)
            nc.vector.tensor_add(t1[:], t1[:], tmp[:])
            nc.vector.tensor_copy(vsb[:, :, tt, 0:64],
                                  qkvb[:, 640:768].rearrange("p (kv c) -> p kv c",
                                                             kv=2))
            return t1

        prevq = []
        def qkv_kp(pq, xtile, kp):
            # exact-ish fp8 hi/lo split: (xh+xl)(wh+wl) ~ xh wh + xl wh + xh wl
            # per chunk-pair: 3 DoubleRow matmuls (0.75x bf16 cost); xl*wl
            # dropped
            xh = xtile[:, 2 * kp:2 * kp + 2, 0, :]
            xl = xtile[:, 2 * kp:2 * kp + 2, 1, :]
            for c0, c1 in ((0, 512), (512, 768)):
                wh = wall[:, 2 * kp:2 * kp + 2, 0, c0:c1]
                wl = wall[:, 2 * kp:2 * kp + 2, 1, c0:c1]
                nc.tensor.matmul(pq[:, c0:c1], xh, wh, start=(kp == 0),
                                 stop=False, perf_mode=DR)
                nc.tensor.matmul(pq[:, c0:c1], xl, wh, start=False, stop=False,
                                 perf_mode=DR)
                nc.tensor.matmul(pq[:, c0:c1], xh, wl, start=False,
                                 stop=(kp == KC // 2 - 1), perf_mode=DR)

        for tt in range(NP1):
            if tt + 2 < NTT:
                xfetch(tt + 2)
            if tt in (3, 5, 7, 9):
                w4 = tt // 2 - 1
                nc.sync.dma_start(woth[:, w4, :], wothb[:, w4, :])
            elif tt in (4, 6, 8, 10):
                w4 = tt // 2 - 2
                nc.sync.dma_start(wotl[:, w4, :], wotlb[:, w4, :])
            xtile = xtiles.pop(tt)
            pq = pp_qkv.tile([128, 1024], F32, tag="pq", name=f"pq{tt}")
            for kp in range(KC // 2):
                qkv_kp(pq, xtile, kp)
            if len(prevq) >= 2:
                p1_tail(*prevq.pop(0))
            qkvb = rp.tile([128, 768], BF16, tag="qkvb", name=f"qkvb{tt}")
            nc.scalar.activation(qkvb[:], pq[:, 0:768],
                                 mybir.ActivationFunctionType.Copy,
                                 scale=1.0 / 64.0)
            qk8 = emit_rope(tt, qkvb)
            prevq.append((tt, qk8))
            if NTT >= 16:
                for pit, ph in PRE.get(tt, ()):
                    emit_scores(pit, ph, pp_pre, 2)
                    prescored.add((pit, ph))
                if tt in PREP and tt != NP1 - 1:
                    pit, ph = PREP[tt]
                    # off-diag pairs of it=2's first heads: psum via the pq
                    # ring (freed by this tile's eviction just above)
                    for t in score_group_thunks(pit, ph, pp_qkv, 2,
                                                paired=True, only="off",
                                                ptag="pq"):
                        t()
                    partials.add((pit, ph))
        p1_tail(*prevq.pop(0))
        if NTT >= 16:
            pit, ph = PREP[NP1 - 1]
            for t in score_group_thunks(pit, ph, pp_qkv, 2, paired=True,
                                        only="off", ptag="pq"):
                t()
            partials.add((pit, ph))
        for pv in prevq:
            p1_tail(*pv)
        xfetch(NTT - 2)
        xfetch(NTT - 1)
        pre_pool.close()
        pq_pool.close()

        # ================= phase 2+3: attention + o-proj =================
        pp_att = ctx.enter_context(tc.tile_pool(name="patt", bufs=1, space="PSUM"))
        avs = [None] * 2
        atbs = {}
        atqs = {}
        oproj_q = []

        # deferred tiles: QKV in two po-ring halves + rope + transposes,
        # run as it=0 filler
        dqkvbs = {}
        dqk8s = {}

        def deferred_thunks(tt):
            dpqs = {}

            def qkv_half(half, part):
                def f():
                    c0, c1 = (0, 512) if half == 0 else (512, 768)
                    xtile = xtiles[tt]
                    if part == 0:
                        dpqs[half] = pp_att.tile([128, c1 - c0], F32,
                                                 tag="po", bufs=2,
                                                 name=f"dpq{tt}_{half}")
                    dpq = dpqs[half]
                    kps = range(4 * part, 4 * part + 4)
                    for kp in kps:
                        xh = xtile[:, 2 * kp:2 * kp + 2, 0, :]
                        xl = xtile[:, 2 * kp:2 * kp + 2, 1, :]
                        wh = wall[:, 2 * kp:2 * kp + 2, 0, c0:c1]
                        wl = wall[:, 2 * kp:2 * kp + 2, 1, c0:c1]
                        nc.tensor.matmul(dpq[:], xh, wh, start=(kp == 0),
                                         stop=False, perf_mode=DR)
                        nc.tensor.matmul(dpq[:], xl, wh, start=False,
                                         stop=False, perf_mode=DR)
                        nc.tensor.matmul(dpq[:], xh, wl, start=False,
                                         stop=(kp == KC // 2 - 1), perf_mode=DR)
                    if part == 0:
                        return
                    if half == 0:
                        dqkvbs[tt] = rp.tile([128, 768], BF16, tag="qkvb",
                                             name=f"qkvb{tt}")
                    else:
                        xtiles.pop(tt)
                    nc.scalar.activation(dqkvbs[tt][:, c0:c1], dpq[:],
                                         mybir.ActivationFunctionType.Copy,
                                         scale=1.0 / 64.0)
                return f

            def rope_f():
                dqk8s[tt] = emit_rope(tt, dqkvbs.pop(tt))

            def tail_f():
                tsl = slice(tt * 128, (tt + 1) * 128)
                qk8 = dqk8s.pop(tt)
                qtr = pp_att.tile([64, 8, 128], BF16, tag="po", bufs=2,
                                  name=f"dqtr{tt}")
                for h in range(8):
                    nc.tensor.matmul(qtr[:, h, :], qk8[:, h * 64:(h + 1) * 64],
                                     idb[:], is_transpose=True,
                                     start=(h == 0), stop=(h == 7))
                nc.vector.tensor_copy(qt[:, :, tsl], qtr[:])
                ktr = pp_att.tile([64, 8, 128], BF16, tag="po", bufs=2,
                                  name=f"dktr{tt}")
                for g in range(2):
                    nc.tensor.matmul(ktr[:, g, :],
                                     qk8[:, 512 + g * 64:512 + (g + 1) * 64],
                                     idb[:], is_transpose=True,
                                     start=(g == 0), stop=(g == 1))
                nc.vector.tensor_copy(kt[:, :, tsl], ktr[:, 0:2, :])

            return [qkv_half(0, 0), qkv_half(0, 1), qkv_half(1, 0),
                    qkv_half(1, 1), rope_f, tail_f]

        deferred = []
        for dtt in range(NP1, NTT):
            deferred += deferred_thunks(dtt)

        def drain_thunk():
            if oproj_q:
                oproj_q.pop(0)()

        carry = [drain_thunk, drain_thunk]
        # it order: ACT-heavy its first (deferred tiles + drains fill under
        # them), prescored/ACT-free its last (absorb the o-proj drains).
        IT_ORDER = [2, 3, 1, 0] if NIT == 4 else list(range(NIT))
        for it in IT_ORDER:
            i0 = it * 512

            def av_thunks(it, h, tqs=(0, 1, 2, 3)):
                kv = h // 4
                pb = pbs[h]
                out = []
                for tq in tqs:
                    def tqf(tq=tq, h=h, kv=kv, pb=pb, it=it):
                        tt = 4 * it + tq
                        if h % 2 == 0 and tq % 2 == 0:
                            avs[tq // 2] = pp_att.tile([128, 2, 2, 128], F32,
                                                       tag=f"avs{tq // 2}",
                                                       bufs=1,
                                                       name=f"avs{it}_{h}_{tq}")
                        slab = avs[tq // 2]
                        for kb in range(tt + 1):
                            nc.tensor.matmul(slab[:, tq % 2, h % 2, 0:65],
                                             pb[:, kb, tq * 128:(tq + 1) * 128],
                                             vsb[:, kv, kb, :],
                                             start=(kb == 0), stop=(kb == tt),
                                             skip_group_check=True)
                        if h % 2 == 1 and tq % 2 == 1:
                            # whole slab (2 tq x 2 h) complete: one fused DVE
                            # normalize for the pair (attr transposes later)
                            g = h // 2
                            sl2 = avs[tq // 2]
                            rec = sm.tile([128, 2, 2, 1], F32, tag="rec")
                            nc.vector.reciprocal(rec[:, :, :, 0],
                                                 sl2[:, :, :, 64])
                            atq = aq.tile([128, 2, 2, 64], BF16, tag="atq",
                                          name=f"atq{it}_{g}_{tq}")
                            nc.vector.tensor_mul(
                                atq[:], sl2[:, :, :, 0:64],
                                rec[:].to_broadcast([128, 2, 2, 64]))
                            atqs[(g, tq // 2)] = atq
                    out.append(tqf)
                return out

            def norm_thunks(it, g, tqs=(0, 1, 2, 3)):
                out = []
                for tq in tqs:
                    def tqf(tq=tq, g=g, it=it):
                        if it not in atbs:
                            atbs[it] = aq.tile([128, 4, 512], BF16, tag="atb",
                                               bufs=1, name=f"atb{it}")
                        atq2 = atqs[(g, tq // 2)]
                        if tq % 2 == 1:
                            atqs.pop((g, tq // 2))
                        attr = pp_att.tile([128, 128], BF16, tag="po", bufs=2,
                                           name=f"attr{it}_{g}_{tq}")
                        nc.tensor.matmul(attr[:],
                                         atq2[:, tq % 2].rearrange(
                                             "p f c -> p (f c)"),
                                         idb[:], is_transpose=True,
                                         start=True, stop=True)
                        nc.vector.tensor_copy(
                            atbs[it][:, g, tq * 128:(tq + 1) * 128], attr[:])
                    out.append(tqf)

                def split_chunk(g=g, it=it, tqs=tqs):
                    # atb chunk (bf16, 16*at) -> ath/atl fp8 for DR o-proj
                    lo, hi = tqs[0] * 128, (tqs[-1] + 1) * 128
                    tsl = slice(it * 512 + lo, it * 512 + hi)
                    atb = atbs[it]
                    nc.gpsimd.tensor_copy(ath[:, g, tsl], atb[:, g, lo:hi])
                    rtmp = aq.tile([128, hi - lo], BF16, tag="rtmp", bufs=2,
                                   name=f"rtmp{it}_{g}_{tqs[0]}")
                    nc.vector.tensor_sub(rtmp[:], atb[:, g, lo:hi],
                                         ath[:, g, tsl])
                    nc.gpsimd.tensor_copy(atl[:, g, tsl], rtmp[:])
                out.append(split_chunk)
                return out

            def queue_oproj_thunk(it_, tts=(0, 1, 2, 3)):
                # drains for its processed late run in the ACT-idle tail:
                # route their psum evictions to ACT, keeping DVE (tail
                # bottleneck) free; early its evict on DVE (ACT saturated).
                act_evict = it_ in (1, 0)

                def qt():
                    state = {}
                    for tq in tts:
                        tt = 4 * it_ + tq
                        for nt in range(4):
                            def step(tt=tt, nt=nt):
                                tsl = slice(tt * 128, (tt + 1) * 128)
                                if nt == 0:
                                    state[tt] = ob.tile([128, D], BF16,
                                                        tag="osb",
                                                        name=f"osb{tt}")
                                osb = state[tt]
                                nsl = slice(nt * 512, (nt + 1) * 512)
                                po = pp_att.tile([128, 512], F32, tag="po",
                                                 bufs=2, name=f"po{tt}_{nt}")
                                k = 0
                                for c2 in range(2):
                                    cs = slice(2 * c2, 2 * c2 + 2)
                                    for a, w in ((ath, woth), (atl, woth),
                                                 (ath, wotl)):
                                        nc.tensor.matmul(
                                            po[:], a[:, cs, tsl], w[:, cs, nsl],
                                            start=(k == 0), stop=(k == 5),
                                            perf_mode=DR)
                                        k += 1
                                if act_evict:
                                    nc.scalar.activation(
                                        osb[:, nsl], po[:],
                                        mybir.ActivationFunctionType.Copy,
                                        scale=1.0 / 1024.0)
                                else:
                                    nc.vector.tensor_scalar_mul(
                                        osb[:, nsl], po[:], 1.0 / 1024.0)
                                nc.sync.dma_start(o[tsl, nsl], osb[:, nsl])
                            oproj_q.append(step)
                return qt

            def run_heads(it, tqs, scores_on):
                nonlocal carry
                for h in range(NH):
                    # filler: PE work whose results ACT doesn't gate — runs
                    # between score groups so exp (2.3x slower than the
                    # score matmul) keeps up without stalling PE on the ring.
                    filler = list(carry)
                    carry = []
                    if h > 0:
                        filler += av_thunks(it, h - 1, tqs)
                        filler += [drain_thunk]
                        if h % 2 == 0:
                            filler += norm_thunks(it, h // 2 - 1, tqs)
                        filler += [drain_thunk]
                        if h in (1, 2):
                            filler += [drain_thunk]
                    if deferred:
                        take = 2 if h > 0 else 4
                        filler += deferred[:take]
                        del deferred[:take]
                    if (it, h) in prescored or not scores_on:
                        pbs[h] = pre_pbs[(it, h)]
                        sc = []
                    elif (it, h) in partials:
                        sc = score_group_thunks(it, h, pp_att, 2, paired=True,
                                                only="diag")
                    else:
                        sc = score_group_thunks(it, h, pp_att, 2, paired=True)
                    # weighted riffle: spread filler evenly across groups
                    if not sc:
                        for t in filler:
                            t()
                    else:
                        fi = 0
                        for i, t in enumerate(sc):
                            t()
                            want = (len(filler) * (i + 1)) // len(sc)
                            while fi < want:
                                filler[fi]()
                                fi += 1
                        while fi < len(filler):
                            filler[fi]()
                            fi += 1
                carry = av_thunks(it, NH - 1, tqs) + [drain_thunk]
                carry += norm_thunks(it, NH // 2 - 1, tqs)

            run_heads(it, (0, 1, 2, 3), True)
            carry += [queue_oproj_thunk(it)]
        for t in carry:
            t()
        while oproj_q:
            oproj_q.pop(0)()

        if dbg:
            dsc = ctx.enter_context(tc.tile_pool(name="dsc", bufs=1))
            for nm, t in [("d_qt", qt[:]), ("d_kt", kt[:]), ("d_vsb", vsb[:]),
                          ("d_at", ath[:]), ("d_pb", pbs[0][:])]:
                f = dsc.tile(list(t.shape), F32, tag="f" + nm, name="f" + nm)
                nc.vector.tensor_copy(f[:], t)
                nc.sync.dma_start(dbg_d[nm][tuple(slice(None) for _ in t.shape)], f[:])
    nc.compile()
    return nc


# ====================== host side ======================

def _fp8(x):
    return np.asarray(x, np.float32).astype(ml_dtypes.float8_e4m3)


def host_inputs(x, Wq, Wk, Wv, Wo, S=2048):
    NTT = S // 128
    KC = D // 128
    inv = ROPE_BASE ** (-np.arange(0, DH, 2, dtype=np.float64) / DH)
    th = np.arange(S, dtype=np.float64)[:, None] * inv[None, :]
    cos1 = np.repeat(np.cos(th), 2, axis=1)
    sin1 = np.sin(th)
    sinsg1 = np.empty((S, 64))
    sinsg1[:, 0::2] = -sin1
    sinsg1[:, 1::2] = sin1
    cosb = np.ascontiguousarray(
        cos1.reshape(NTT, 128, 64).transpose(1, 0, 2)).astype(ml_dtypes.bfloat16)
    sing = np.ascontiguousarray(
        sinsg1.reshape(NTT, 128, 64).transpose(1, 0, 2)).astype(ml_dtypes.bfloat16)
    identf = np.eye(128, dtype=np.float32)
    p = np.arange(128)[:, None]
    q = np.arange(128)[None, :]
    trif = np.where(p <= q, 0.0, MASKVAL).astype(np.float32)

    in_maps = []
    for c in range(NCORES):
        b, kvp = c // 4, c % 4
        xb = np.asarray(x[b], np.float32)
        xh = _fp8(xb)
        xl = _fp8(xb - xh.astype(np.float32))
        xt = np.stack([xh, xl], axis=0).reshape(2, NTT, 128, KC, 128)
        xt8 = np.ascontiguousarray(xt.transpose(1, 4, 3, 0, 2))
        wq = Wq[512 * kvp:512 * (kvp + 1)]
        wk = Wk[128 * kvp:128 * (kvp + 1)]
        wv = Wv[128 * kvp:128 * (kvp + 1)]
        wall = np.concatenate([wq, wk, wv], axis=0) * 64.0
        wh = _fp8(wall)
        wl = _fp8(wall - wh.astype(np.float32))
        wall8 = np.ascontiguousarray(
            np.stack([wh, wl], axis=0).transpose(2, 0, 1)
            .reshape(KC, 128, 2, 768).transpose(1, 0, 2, 3))
        wo64 = np.ascontiguousarray(
            (Wo[:, 512 * kvp:512 * (kvp + 1)] * 64.0).astype(np.float32)
            .T.reshape(4, 128, D).transpose(1, 0, 2))
        woh = _fp8(wo64)
        wol = _fp8(wo64 - woh.astype(np.float32))
        in_maps.append(dict(xt8=xt8, wall8=wall8, wothb=woh, wotlb=wol,
                            cosb=cosb, sinsg=sing, identf=identf, trif=trif))
    return in_maps


def kernel(**inputs):
    x = np.asarray(inputs["x"], dtype=np.float32)
    Wq = np.asarray(inputs["Wq"], dtype=np.float32)
    Wk = np.asarray(inputs["Wk"], dtype=np.float32)
    Wv = np.asarray(inputs["Wv"], dtype=np.float32)
    Wo = np.asarray(inputs["Wo"], dtype=np.float32)
    B, S, _ = x.shape
    in_maps = host_inputs(x, Wq, Wk, Wv, Wo, S=S)
    if "nc" not in _cached:
        _cached["nc"] = build_nc(S=S)
    res = run_bass_kernel_spmd(_cached["nc"], in_maps, list(range(NCORES)))
    out = np.zeros((B, S, D), np.float64)
    for c, r in enumerate(res.results):
        out[c // 4] += np.asarray(r["o"], np.float32)
    return out.astype(np.float32)



# revision 78
# speedup vs baseline: 1.0113x; 1.0113x over previous
"""GQA (32q/8kv heads, RoPE, causal) TRN2 kernel v4.

Sharding: 8 cores = 2 batches x 4 kv-pairs. Core (b, kvp) owns batch b,
kv heads {2kvp, 2kvp+1}, q heads 8kvp..8kvp+7. Each core emits a partial
o [S, D] (bf16); host sums 4 partials per batch.

Precision: QKV projection and o-proj in fp8e4 DoubleRow hi/lo 3-term
(0.75x bf16 cost, ~2^-9 rel error). Scores/probs/AV in bf16 (contraction
64 / the hi/lo slots can't double there, so fp8 buys nothing in the
ap-size cost model). o-proj operands scaled (at x16 via a 1/16 ones-col,
Wo x64) to keep fp8-hi in the normal range; the osb evict divides by 1024.

Schedule (the cost model charges matmuls output-free-size only; exp on ACT
costs 2.3x the score matmul that feeds it, so ACT is the phase-2 constraint):
  phase 1 (13 tiles): split weight/x DMAs so QKV starts ~4.4us in; QKV fp8
    DR -> ACT evicts (Copy, 1/64) -> RoPE (2 DVE muls + in-place add) ->
    PE transposes -> qt/kt; it=0 fully prescored (exp'd) under QKV.
  phase 2, it order (2,3,1,0): per head a weighted riffle interleaves
    score-pair groups ([128,2,512] psum, one wide exp each) with filler:
    AV of the previous head (65-wide DR-free accumulation into 2 slab
    banks + fused slab normalize on DVE), at-transposes, per-chunk fp8
    hi/lo splits (Pool cast + DVE sub + Pool cast), o-proj drains of the
    previous it (6 DR matmuls/step), and tiles 13-15's whole pipeline
    (deferred into the ACT-bound it=2 window, psum via the po ring).
    ACT-free its (1, 0) run last, absorbing drains; their o evictions go
    through ACT (idle there) instead of DVE; o DMA per 512-col chunk.
"""
import numpy as np
from contextlib import ExitStack

import concourse.bass as bass
from concourse import bacc
import concourse.mybir as mybir
import concourse.tile as tile
from concourse.bass_utils import run_bass_kernel_spmd
import ml_dtypes

F32 = mybir.dt.float32
BF16 = mybir.dt.bfloat16
FP8 = mybir.dt.float8e4
EXP = mybir.ActivationFunctionType.Exp
DR = mybir.MatmulPerfMode.DoubleRow

D = 2048
DH = 64
NCORES = 8
ROPE_BASE = 10000.0
MASKVAL = -240.0
EBIAS = -2.0     # probs = exp(0.125*scores - 2); cancels in normalization

_cached = {}


def build_nc(S=2048, dbg=False):
    NTT = S // 128
    NIT = S // 512
    KC = D // 128
    NH = 8
    nc = bacc.Bacc("TRN2", target_bir_lowering=False, debug=False)
    dbg_d = {}
    if dbg:
        dbg_d["d_qt"] = nc.declare_dram_parameter("d_qt", [64, NH, S], F32, isOutput=True)
        dbg_d["d_kt"] = nc.declare_dram_parameter("d_kt", [64, 2, S], F32, isOutput=True)
        dbg_d["d_vsb"] = nc.declare_dram_parameter("d_vsb", [128, 2, NTT, 65], F32, isOutput=True)
        dbg_d["d_at"] = nc.declare_dram_parameter("d_at", [128, 4, S], F32, isOutput=True)
        dbg_d["d_pb"] = nc.declare_dram_parameter("d_pb", [128, NTT, 512], F32, isOutput=True)
    xt8 = nc.declare_dram_parameter("xt8", [NTT, 128, KC, 2, 128], FP8, isOutput=False)
    wall8 = nc.declare_dram_parameter("wall8", [128, KC, 2, 768], FP8, isOutput=False)
    wothb = nc.declare_dram_parameter("wothb", [128, 4, D], FP8, isOutput=False)
    wotlb = nc.declare_dram_parameter("wotlb", [128, 4, D], FP8, isOutput=False)
    cosb = nc.declare_dram_parameter("cosb", [128, NTT, 64], BF16, isOutput=False)
    sinsg = nc.declare_dram_parameter("sinsg", [128, NTT, 64], BF16, isOutput=False)
    identf = nc.declare_dram_parameter("identf", [128, 128], F32, isOutput=False)
    trif = nc.declare_dram_parameter("trif", [128, 128], F32, isOutput=False)
    o = nc.declare_dram_parameter("o", [S, D], BF16, isOutput=True)

    with tile.TileContext(nc) as tc, ExitStack() as ctx:
        wp = ctx.enter_context(tc.tile_pool(name="weights", bufs=1))
        sp = ctx.enter_context(tc.tile_pool(name="state", bufs=1))
        xs = ctx.enter_context(tc.tile_pool(name="xstream", bufs=5))
        rp = ctx.enter_context(tc.tile_pool(name="ring", bufs=2))
        pr = ctx.enter_context(tc.tile_pool(name="probs", bufs=3))
        aq = ctx.enter_context(tc.tile_pool(name="atq", bufs=2))
        ob = ctx.enter_context(tc.tile_pool(name="osb", bufs=2))
        sm = ctx.enter_context(tc.tile_pool(name="small", bufs=2))

        # ---------- persistent weights / tables ----------
        wall = wp.tile([128, KC, 2, 768], FP8, tag="wall")
        woth = wp.tile([128, 4, D], FP8, tag="woth")
        wotl = wp.tile([128, 4, D], FP8, tag="wotl")
        cos_sb = wp.tile([128, NTT, 64], BF16, tag="cos")
        sin_sb = wp.tile([128, NTT, 64], BF16, tag="sin")
        idb = wp.tile([128, 128], BF16, tag="idb")
        trib = wp.tile([128, 128], BF16, tag="trib")
        nbias = wp.tile([128, 1], F32, tag="nbias")

        # DMA issue order == execution order (single queue): x tiles first so
        # QKV can start ~4us in, then tables (needed by tile-0 rope), then
        # wall in per-chunk-pair pieces (matmul kp waits only on its chunk),
        # wot split across the loop (first needed ~90us in).
        xtiles = {}

        def xfetch(tt):
            t = xs.tile([128, KC, 2, 128], FP8, tag="xt", name=f"xt{tt}")
            nc.sync.dma_start(t[:], xt8[tt])
            xtiles[tt] = t

        def wfetch(kp):
            nc.sync.dma_start(wall[:, 2 * kp:2 * kp + 2, :, :],
                              wall8[:, 2 * kp:2 * kp + 2, :, :])

        xfetch(0)
        wfetch(0)
        wfetch(1)
        xfetch(1)
        nc.sync.dma_start(cos_sb[:], cosb[:, :, :])
        nc.sync.dma_start(sin_sb[:], sinsg[:, :, :])
        wfetch(2)
        idf_s = sm.tile([128, 128], F32, tag="idf")
        trf_s = sm.tile([128, 128], F32, tag="trf")
        nc.sync.dma_start(idf_s[:], identf[:, :])
        nc.sync.dma_start(trf_s[:], trif[:, :])
        for kp in range(3, KC // 2):
            wfetch(kp)
        nc.vector.tensor_copy(idb[:], idf_s[:])
        nc.vector.tensor_copy(trib[:], trf_s[:])
        nc.vector.memset(nbias[:], EBIAS)

        # ---------- per-core state ----------
        qt = sp.tile([64, NH, S], BF16, tag="qt")
        kt = sp.tile([64, 2, S], BF16, tag="kt")
        vsb = sp.tile([128, 2, NTT, 65], BF16, tag="vsb")
        # at held as fp8 hi/lo (o-proj runs in fp8 DoubleRow); atq carries
        # 16*at (ones-col = 1/16 makes rec = 16/denom) so the fp8-hi of small
        # at entries stays in the normal range; o eviction divides by 16*64.
        ath = sp.tile([128, 4, S], FP8, tag="ath")
        atl = sp.tile([128, 4, S], FP8, tag="atl")
        nc.vector.memset(vsb[:, :, :, 64:65], 1.0 / 16.0)

        # ================= phase 1: QKV + rope + transposes =================
        pq_pool = ExitStack()
        pp_qkv = pq_pool.enter_context(tc.tile_pool(name="pqkv", bufs=2, space="PSUM"))
        pp_tr = pq_pool.enter_context(tc.tile_pool(name="ptr", bufs=2, space="PSUM"))
        pre_pool = ExitStack()
        pp_pre = pre_pool.enter_context(tc.tile_pool(name="presc", bufs=1, space="PSUM"))

        pbs = [None] * NH
        pre_pbs = {}
        prescored = set()

        def score_group_thunks(it, h, pool, scbufs, paired, only="all",
                               ptag="sc2"):
            """Allocate pb for (it, h), return one thunk per psum score group
            (off-diag pair or diag single). only="off"/"diag" emits just that
            subset ("diag" reuses the pb allocated by an earlier "off" call —
            partial prescoring)."""
            kv = h // 4
            nkb = 4 * it + 4
            i0 = it * 512
            if only == "diag":
                pb = pre_pbs[(it, h)]
                pbs[h] = pb
            else:
                tag = "probs0" if (it == 0 and NTT >= 16) else "probs"
                pbufs = 6 if tag == "probs0" else 2
                pb = pr.tile([128, nkb, 512], BF16, tag=tag, bufs=pbufs,
                             name=f"pb{it}_{h}")
                pbs[h] = pb
                pre_pbs[(it, h)] = pb
            ndiag = 4 * it
            if paired:
                groups = []
                if only in ("all", "off"):
                    groups += [(kb, kb + 1) for kb in range(0, ndiag, 2)]
                if only in ("all", "diag"):
                    groups += [(kb,) for kb in range(ndiag, nkb)]
            else:
                groups = [(kb,) for kb in range(nkb)]

            def emit_group(grp):
                if len(grp) == 2:
                    # off-diagonal pair: 2 one-bank psum slots, one wide exp
                    p2 = pool.tile([128, 2, 512], F32, tag=ptag, bufs=scbufs,
                                   name=f"psc{it}_{h}_{grp[0]}p")
                    for j, kb in enumerate(grp):
                        nc.tensor.matmul(p2[:, j, :],
                                         kt[:, kv, kb * 128:(kb + 1) * 128],
                                         qt[:, h, i0:i0 + 512],
                                         start=True, stop=True)
                    nc.scalar.activation(pb[:, grp[0]:grp[0] + 2, :], p2[:],
                                         EXP, scale=0.125, bias=nbias[:])
                    return
                kb = grp[0]
                diag = kb >= ndiag
                qlo = (kb - ndiag) * 128 if diag else 0
                psc = pool.tile([128, 512], F32, tag="sc2", bufs=scbufs,
                                name=f"psc{it}_{h}_{kb}")
                nc.tensor.matmul(psc[:, qlo:512],
                                 kt[:, kv, kb * 128:(kb + 1) * 128],
                                 qt[:, h, i0 + qlo:i0 + 512],
                                 start=True, stop=not diag)
                if diag:
                    nc.tensor.matmul(psc[:, qlo:qlo + 128], idb[:], trib[:],
                                     start=False, stop=True)
                nc.scalar.activation(pb[:, kb, qlo:512], psc[:, qlo:512],
                                     EXP, scale=0.125, bias=nbias[:])
                if qlo:
                    nc.gpsimd.memset(pb[:, kb, 0:qlo], 0.0)

            return [lambda grp=grp: emit_group(grp) for grp in groups]

        def emit_scores(it, h, pool, scbufs, paired=False):
            for t in score_group_thunks(it, h, pool, scbufs, paired):
                t()

        def p1_tail(tt, qk8):
            tsl = slice(tt * 128, (tt + 1) * 128)
            qtr = pp_tr.tile([64, 8, 128], BF16, tag="qtr", name=f"qtr{tt}")
            ktr = pp_tr.tile([64, 8, 128], BF16, tag="qtr", name=f"ktr{tt}")
            ktr = ktr[:, 0:2, :]
            for h in range(8):
                nc.tensor.matmul(qtr[:, h, :], qk8[:, h * 64:(h + 1) * 64],
                                 idb[:], is_transpose=True,
                                 start=(h == 0), stop=(h == 7))
            for g in range(2):
                nc.tensor.matmul(ktr[:, g, :],
                                 qk8[:, 512 + g * 64:512 + (g + 1) * 64],
                                 idb[:], is_transpose=True,
                                 start=(g == 0), stop=(g == 1))
            nc.vector.tensor_copy(qt[:, :, tsl], qtr[:])
            nc.vector.tensor_copy(kt[:, :, tsl], ktr[:])

        # Tiles 13..15 are deferred into phase 2 (it=0 is fully prescored, so
        # the PE sits under ACT-idle there; their psum comes from the po ring).
        NDEF = 5 if NTT >= 16 else 0
        NP1 = NTT - NDEF
        PRE = {5: [(0, 0)], 6: [(0, 1)], 7: [(0, 2)], 8: [(0, 3)],
               9: [(0, 4)], 10: [(0, 5)]}
        PREP = {9: (2, 0), 10: (2, 1)}
        partials = set()

        def emit_rope(tt, qkvb):
            # rope: op1 tmp = pairswap(qk)*sinsg (Pool); op2 t1 = qk*cos
            # (Pool); op3 qk8 = t1+tmp in-place (DVE, bf16 2x); + vsb copy
            qkv = qkvb[:, 0:640]
            swp = qkv.rearrange("p (h n two) -> p h n two", two=2, n=32)[..., ::-1]
            tmp = rp.tile([128, 640], BF16, tag="tmp", name=f"tmp{tt}")
            t1 = rp.tile([128, 640], BF16, tag="t1", name=f"t1_{tt}")
            sin4 = sin_sb[:, tt, :].rearrange("p (one n two) -> p one n two",
                                              one=1, two=2).to_broadcast([128, 10, 32, 2])
            cos3 = cos_sb[:, tt, :].rearrange("p (one c) -> p one c",
                                              one=1).to_broadcast([128, 10, 64])
            nc.vector.tensor_mul(tmp[:].rearrange("p (h n two) -> p h n two",
                                                  two=2, n=32),
                                 swp, sin4)
            nc.vector.tensor_mul(t1[:].rearrange("p (h c) -> p h c", h=10),
                                 qkv.rearrange("p (h c) -> p h c", h=10), cos3)
            nc.vector.tensor_add(t1[:], t1[:], tmp[:])
            nc.vector.tensor_copy(vsb[:, :, tt, 0:64],
                                  qkvb[:, 640:768].rearrange("p (kv c) -> p kv c",
                                                             kv=2))
            return t1

        prevq = []
        def qkv_kp(pq, xtile, kp):
            # exact-ish fp8 hi/lo split: (xh+xl)(wh+wl) ~ xh wh + xl wh + xh wl
            # per chunk-pair: 3 DoubleRow matmuls (0.75x bf16 cost); xl*wl
            # dropped
            xh = xtile[:, 2 * kp:2 * kp + 2, 0, :]
            xl = xtile[:, 2 * kp:2 * kp + 2, 1, :]
            for c0, c1 in ((0, 512), (512, 768)):
                wh = wall[:, 2 * kp:2 * kp + 2, 0, c0:c1]
                wl = wall[:, 2 * kp:2 * kp + 2, 1, c0:c1]
                nc.tensor.matmul(pq[:, c0:c1], xh, wh, start=(kp == 0),
                                 stop=False, perf_mode=DR)
                nc.tensor.matmul(pq[:, c0:c1], xl, wh, start=False, stop=False,
                                 perf_mode=DR)
                nc.tensor.matmul(pq[:, c0:c1], xh, wl, start=False,
                                 stop=(kp == KC // 2 - 1), perf_mode=DR)

        for tt in range(NP1):
            if tt + 2 < NTT:
                xfetch(tt + 2)
            if tt in (3, 5, 7, 9):
                w4 = tt // 2 - 1
                nc.sync.dma_start(woth[:, w4, :], wothb[:, w4, :])
            elif tt in (4, 6, 8, 10):
                w4 = tt // 2 - 2
                nc.sync.dma_start(wotl[:, w4, :], wotlb[:, w4, :])
            xtile = xtiles.pop(tt)
            pq = pp_qkv.tile([128, 1024], F32, tag="pq", name=f"pq{tt}")
            for kp in range(KC // 2):
                qkv_kp(pq, xtile, kp)
            if len(prevq) >= 2:
                p1_tail(*prevq.pop(0))
            qkvb = rp.tile([128, 768], BF16, tag="qkvb", name=f"qkvb{tt}")
            nc.scalar.activation(qkvb[:], pq[:, 0:768],
                                 mybir.ActivationFunctionType.Copy,
                                 scale=1.0 / 64.0)
            qk8 = emit_rope(tt, qkvb)
            prevq.append((tt, qk8))
            if NTT >= 16:
                for pit, ph in PRE.get(tt, ()):
                    emit_scores(pit, ph, pp_pre, 2)
                    prescored.add((pit, ph))
                if tt in PREP and tt != NP1 - 1:
                    pit, ph = PREP[tt]
                    # off-diag pairs of it=2's first heads: psum via the pq
                    # ring (freed by this tile's eviction just above)
                    for t in score_group_thunks(pit, ph, pp_qkv, 2,
                                                paired=True, only="off",
                                                ptag="pq"):
                        t()
                    partials.add((pit, ph))
        p1_tail(*prevq.pop(0))
        if NTT >= 16:
            pit, ph = PREP[NP1 - 1]
            for t in score_group_thunks(pit, ph, pp_qkv, 2, paired=True,
                                        only="off", ptag="pq"):
                t()
            partials.add((pit, ph))
        for pv in prevq:
            p1_tail(*pv)
        xfetch(NTT - 3)
        xfetch(NTT - 2)
        xfetch(NTT - 1)
        pre_pool.close()
        pq_pool.close()

        # ================= phase 2+3: attention + o-proj =================
        pp_att = ctx.enter_context(tc.tile_pool(name="patt", bufs=1, space="PSUM"))
        avs = [None] * 2
        atbs = {}
        atqs = {}
        oproj_q = []

        # deferred tiles: QKV in two po-ring halves + rope + transposes,
        # run as it=0 filler
        dqkvbs = {}
        dqk8s = {}

        def deferred_thunks(tt):
            dpqs = {}

            def qkv_half(half, part):
                def f():
                    c0, c1 = (0, 512) if half == 0 else (512, 768)
                    xtile = xtiles[tt]
                    if part == 0:
                        dpqs[half] = pp_att.tile([128, c1 - c0], F32,
                                                 tag="po", bufs=2,
                                                 name=f"dpq{tt}_{half}")
                    dpq = dpqs[half]
                    kps = range(4 * part, 4 * part + 4)
                    for kp in kps:
                        xh = xtile[:, 2 * kp:2 * kp + 2, 0, :]
                        xl = xtile[:, 2 * kp:2 * kp + 2, 1, :]
                        wh = wall[:, 2 * kp:2 * kp + 2, 0, c0:c1]
                        wl = wall[:, 2 * kp:2 * kp + 2, 1, c0:c1]
                        nc.tensor.matmul(dpq[:], xh, wh, start=(kp == 0),
                                         stop=False, perf_mode=DR)
                        nc.tensor.matmul(dpq[:], xl, wh, start=False,
                                         stop=False, perf_mode=DR)
                        nc.tensor.matmul(dpq[:], xh, wl, start=False,
                                         stop=(kp == KC // 2 - 1), perf_mode=DR)
                    if part == 0:
                        return
                    if half == 0:
                        dqkvbs[tt] = rp.tile([128, 768], BF16, tag="qkvb",
                                             name=f"qkvb{tt}")
                    else:
                        xtiles.pop(tt)
                    nc.scalar.activation(dqkvbs[tt][:, c0:c1], dpq[:],
                                         mybir.ActivationFunctionType.Copy,
                                         scale=1.0 / 64.0)
                return f

            def rope_f():
                dqk8s[tt] = emit_rope(tt, dqkvbs.pop(tt))

            def tail_f():
                tsl = slice(tt * 128, (tt + 1) * 128)
                qk8 = dqk8s.pop(tt)
                qtr = pp_att.tile([64, 8, 128], BF16, tag="po", bufs=2,
                                  name=f"dqtr{tt}")
                for h in range(8):
                    nc.tensor.matmul(qtr[:, h, :], qk8[:, h * 64:(h + 1) * 64],
                                     idb[:], is_transpose=True,
                                     start=(h == 0), stop=(h == 7))
                nc.vector.tensor_copy(qt[:, :, tsl], qtr[:])
                ktr = pp_att.tile([64, 8, 128], BF16, tag="po", bufs=2,
                                  name=f"dktr{tt}")
                for g in range(2):
                    nc.tensor.matmul(ktr[:, g, :],
                                     qk8[:, 512 + g * 64:512 + (g + 1) * 64],
                                     idb[:], is_transpose=True,
                                     start=(g == 0), stop=(g == 1))
                nc.vector.tensor_copy(kt[:, :, tsl], ktr[:, 0:2, :])

            return [qkv_half(0, 0), qkv_half(0, 1), qkv_half(1, 0),
                    qkv_half(1, 1), rope_f, tail_f]

        deferred = []
        for dtt in range(NP1, NTT):
            deferred += deferred_thunks(dtt)

        def drain_thunk():
            if oproj_q:
                oproj_q.pop(0)()

        carry = [drain_thunk, drain_thunk]
        # it order: ACT-heavy its first (deferred tiles + drains fill under
        # them), prescored/ACT-free its last (absorb the o-proj drains).
        IT_ORDER = [2, 3, 1, 0] if NIT == 4 else list(range(NIT))
        for it in IT_ORDER:
            i0 = it * 512

            def av_thunks(it, h, tqs=(0, 1, 2, 3)):
                kv = h // 4
                pb = pbs[h]
                out = []
                for tq in tqs:
                    def tqf(tq=tq, h=h, kv=kv, pb=pb, it=it):
                        tt = 4 * it + tq
                        if h % 2 == 0 and tq % 2 == 0:
                            avs[tq // 2] = pp_att.tile([128, 2, 2, 128], F32,
                                                       tag=f"avs{tq // 2}",
                                                       bufs=1,
                                                       name=f"avs{it}_{h}_{tq}")
                        slab = avs[tq // 2]
                        for kb in range(tt + 1):
                            nc.tensor.matmul(slab[:, tq % 2, h % 2, 0:65],
                                             pb[:, kb, tq * 128:(tq + 1) * 128],
                                             vsb[:, kv, kb, :],
                                             start=(kb == 0), stop=(kb == tt),
                                             skip_group_check=True)
                        if h % 2 == 1 and tq % 2 == 1:
                            # whole slab (2 tq x 2 h) complete: one fused DVE
                            # normalize for the pair (attr transposes later)
                            g = h // 2
                            sl2 = avs[tq // 2]
                            rec = sm.tile([128, 2, 2, 1], F32, tag="rec")
                            nc.vector.reciprocal(rec[:, :, :, 0],
                                                 sl2[:, :, :, 64])
                            atq = aq.tile([128, 2, 2, 64], BF16, tag="atq",
                                          name=f"atq{it}_{g}_{tq}")
                            nc.vector.tensor_mul(
                                atq[:], sl2[:, :, :, 0:64],
                                rec[:].to_broadcast([128, 2, 2, 64]))
                            atqs[(g, tq // 2)] = atq
                    out.append(tqf)
                return out

            def norm_thunks(it, g, tqs=(0, 1, 2, 3)):
                out = []
                for tq in tqs:
                    def tqf(tq=tq, g=g, it=it):
                        if it not in atbs:
                            atbs[it] = aq.tile([128, 4, 512], BF16, tag="atb",
                                               bufs=1, name=f"atb{it}")
                        atq2 = atqs[(g, tq // 2)]
                        if tq % 2 == 1:
                            atqs.pop((g, tq // 2))
                        attr = pp_att.tile([128, 128], BF16, tag="po", bufs=2,
                                           name=f"attr{it}_{g}_{tq}")
                        nc.tensor.matmul(attr[:],
                                         atq2[:, tq % 2].rearrange(
                                             "p f c -> p (f c)"),
                                         idb[:], is_transpose=True,
                                         start=True, stop=True)
                        nc.vector.tensor_copy(
                            atbs[it][:, g, tq * 128:(tq + 1) * 128], attr[:])
                    out.append(tqf)

                def split_chunk(g=g, it=it, tqs=tqs):
                    # atb chunk (bf16, 16*at) -> ath/atl fp8 for DR o-proj
                    lo, hi = tqs[0] * 128, (tqs[-1] + 1) * 128
                    tsl = slice(it * 512 + lo, it * 512 + hi)
                    atb = atbs[it]
                    nc.gpsimd.tensor_copy(ath[:, g, tsl], atb[:, g, lo:hi])
                    rtmp = aq.tile([128, hi - lo], BF16, tag="rtmp", bufs=2,
                                   name=f"rtmp{it}_{g}_{tqs[0]}")
                    nc.vector.tensor_sub(rtmp[:], atb[:, g, lo:hi],
                                         ath[:, g, tsl])
                    nc.gpsimd.tensor_copy(atl[:, g, tsl], rtmp[:])
                out.append(split_chunk)
                return out

            def queue_oproj_thunk(it_, tts=(0, 1, 2, 3)):
                # drains for its processed late run in the ACT-idle tail:
                # route their psum evictions to ACT, keeping DVE (tail
                # bottleneck) free; early its evict on DVE (ACT saturated).
                act_evict = it_ in (1, 0)

                def qt():
                    state = {}
                    for tq in tts:
                        tt = 4 * it_ + tq
                        for nt in range(4):
                            def step(tt=tt, nt=nt):
                                tsl = slice(tt * 128, (tt + 1) * 128)
                                if nt == 0:
                                    state[tt] = ob.tile([128, D], BF16,
                                                        tag="osb",
                                                        name=f"osb{tt}")
                                osb = state[tt]
                                nsl = slice(nt * 512, (nt + 1) * 512)
                                po = pp_att.tile([128, 512], F32, tag="po",
                                                 bufs=2, name=f"po{tt}_{nt}")
                                k = 0
                                for c2 in range(2):
                                    cs = slice(2 * c2, 2 * c2 + 2)
                                    for a, w in ((ath, woth), (atl, woth),
                                                 (ath, wotl)):
                                        nc.tensor.matmul(
                                            po[:], a[:, cs, tsl], w[:, cs, nsl],
                                            start=(k == 0), stop=(k == 5),
                                            perf_mode=DR)
                                        k += 1
                                if act_evict:
                                    nc.scalar.activation(
                                        osb[:, nsl], po[:],
                                        mybir.ActivationFunctionType.Copy,
                                        scale=1.0 / 1024.0)
                                else:
                                    nc.vector.tensor_scalar_mul(
                                        osb[:, nsl], po[:], 1.0 / 1024.0)
                                nc.sync.dma_start(o[tsl, nsl], osb[:, nsl])
                            oproj_q.append(step)
                return qt

            def run_heads(it, tqs, scores_on):
                nonlocal carry
                for h in range(NH):
                    # filler: PE work whose results ACT doesn't gate — runs
                    # between score groups so exp (2.3x slower than the
                    # score matmul) keeps up without stalling PE on the ring.
                    filler = list(carry)
                    carry = []
                    if h > 0:
                        filler += av_thunks(it, h - 1, tqs)
                        filler += [drain_thunk]
                        if h % 2 == 0:
                            filler += norm_thunks(it, h // 2 - 1, tqs)
                        filler += [drain_thunk]
                        if h in (1, 2):
                            filler += [drain_thunk]
                    if deferred:
                        take = 2 if h > 0 else 4
                        filler += deferred[:take]
                        del deferred[:take]
                    if (it, h) in prescored or not scores_on:
                        pbs[h] = pre_pbs[(it, h)]
                        sc = []
                    elif (it, h) in partials:
                        sc = score_group_thunks(it, h, pp_att, 2, paired=True,
                                                only="diag")
                    else:
                        sc = score_group_thunks(it, h, pp_att, 2, paired=True)
                    # weighted riffle: spread filler evenly across groups
                    if not sc:
                        for t in filler:
                            t()
                    else:
                        fi = 0
                        for i, t in enumerate(sc):
                            t()
                            want = (len(filler) * (i + 1)) // len(sc)
                            while fi < want:
                                filler[fi]()
                                fi += 1
                        while fi < len(filler):
                            filler[fi]()
                            fi += 1
                carry = av_thunks(it, NH - 1, tqs) + [drain_thunk]
                carry += norm_thunks(it, NH // 2 - 1, tqs)

            run_heads(it, (0, 1, 2, 3), True)
            carry += [queue_oproj_thunk(it)]
        for t in carry:
            t()
        while oproj_q:
            oproj_q.pop(0)()

        if dbg:
            dsc = ctx.enter_context(tc.tile_pool(name="dsc", bufs=1))
            for nm, t in [("d_qt", qt[:]), ("d_kt", kt[:]), ("d_vsb", vsb[:]),
                          ("d_at", ath[:]), ("d_pb", pbs[0][:])]:
                f = dsc.tile(list(t.shape), F32, tag="f" + nm, name="f" + nm)
                nc.vector.tensor_copy(f[:], t)
                nc.sync.dma_start(dbg_d[nm][tuple(slice(None) for _ in t.shape)], f[:])
    nc.compile()
    return nc


# ====================== host side ======================

def _fp8(x):
    return np.asarray(x, np.float32).astype(ml_dtypes.float8_e4m3)


def host_inputs(x, Wq, Wk, Wv, Wo, S=2048):
    NTT = S // 128
    KC = D // 128
    inv = ROPE_BASE ** (-np.arange(0, DH, 2, dtype=np.float64) / DH)
    th = np.arange(S, dtype=np.float64)[:, None] * inv[None, :]
    cos1 = np.repeat(np.cos(th), 2, axis=1)
    sin1 = np.sin(th)
    sinsg1 = np.empty((S, 64))
    sinsg1[:, 0::2] = -sin1
    sinsg1[:, 1::2] = sin1
    cosb = np.ascontiguousarray(
        cos1.reshape(NTT, 128, 64).transpose(1, 0, 2)).astype(ml_dtypes.bfloat16)
    sing = np.ascontiguousarray(
        sinsg1.reshape(NTT, 128, 64).transpose(1, 0, 2)).astype(ml_dtypes.bfloat16)
    identf = np.eye(128, dtype=np.float32)
    p = np.arange(128)[:, None]
    q = np.arange(128)[None, :]
    trif = np.where(p <= q, 0.0, MASKVAL).astype(np.float32)

    in_maps = []
    for c in range(NCORES):
        b, kvp = c // 4, c % 4
        xb = np.asarray(x[b], np.float32)
        xh = _fp8(xb)
        xl = _fp8(xb - xh.astype(np.float32))
        xt = np.stack([xh, xl], axis=0).reshape(2, NTT, 128, KC, 128)
        xt8 = np.ascontiguousarray(xt.transpose(1, 4, 3, 0, 2))
        wq = Wq[512 * kvp:512 * (kvp + 1)]
        wk = Wk[128 * kvp:128 * (kvp + 1)]
        wv = Wv[128 * kvp:128 * (kvp + 1)]
        wall = np.concatenate([wq, wk, wv], axis=0) * 64.0
        wh = _fp8(wall)
        wl = _fp8(wall - wh.astype(np.float32))
        wall8 = np.ascontiguousarray(
            np.stack([wh, wl], axis=0).transpose(2, 0, 1)
            .reshape(KC, 128, 2, 768).transpose(1, 0, 2, 3))
        wo64 = np.ascontiguousarray(
            (Wo[:, 512 * kvp:512 * (kvp + 1)] * 64.0).astype(np.float32)
            .T.reshape(4, 128, D).transpose(1, 0, 2))
        woh = _fp8(wo64)
        wol = _fp8(wo64 - woh.astype(np.float32))
        in_maps.append(dict(xt8=xt8, wall8=wall8, wothb=woh, wotlb=wol,
                            cosb=cosb, sinsg=sing, identf=identf, trif=trif))
    return in_maps


def kernel(**inputs):
    x = np.asarray(inputs["x"], dtype=np.float32)
    Wq = np.asarray(inputs["Wq"], dtype=np.float32)
    Wk = np.asarray(inputs["Wk"], dtype=np.float32)
    Wv = np.asarray(inputs["Wv"], dtype=np.float32)
    Wo = np.asarray(inputs["Wo"], dtype=np.float32)
    B, S, _ = x.shape
    in_maps = host_inputs(x, Wq, Wk, Wv, Wo, S=S)
    if "nc" not in _cached:
        _cached["nc"] = build_nc(S=S)
    res = run_bass_kernel_spmd(_cached["nc"], in_maps, list(range(NCORES)))
    out = np.zeros((B, S, D), np.float64)
    for c, r in enumerate(res.results):
        out[c // 4] += np.asarray(r["o"], np.float32)
    return out.astype(np.float32)

